# revision 1
# baseline (speedup 1.0000x reference)
"""Trainium2 Bass kernel for fused Llama attention (nn_LlamaAttentionFused).

Reference computation (B=2, S=1024, H=4096, 32 Q heads, 8 KV heads, D=128):
    xq = x @ wq; xk = x @ wk; xv = x @ wv
    rope(xq, xk); causal GQA flash attention; out = attn @ wo

Sharding: 8-way tensor parallel over heads. Core c owns Q heads 4c..4c+3 and
KV head c (GQA groups stay together), i.e. columns [512c, 512c+512) of wq,
columns [128c, 128c+128) of wk/wv, and rows [512c, 512c+512) of wo. Each core
computes a full-shape partial output (its heads' contribution through wo);
the host sums the 8 partials.

All matmuls run as float32r (full-rate fp32 on the PE when the moving free
dim >= 256). Softmax is exact (row max subtraction + renormalization).

Device-side layouts (per core):
    xT   [4096, 2048]  x transposed on host (tokens = 2 batches x 1024)
    wq   [4096, 512]   natural (used as stationary [K=H, M=dims])
    wkv  [4096, 256]   wk|wv column-concat
    wo   [512, 4096]   natural (moving operand)
    cosf/sinf [128, 1024]  freqs_cos.T / freqs_sin.T stacked twice on the
                           partition axis (RoPE needs them on both halves)
    out  [2048, 4096]  partial output
"""

import numpy as np

import concourse.bass as bass
import concourse.mybir as mybir
import concourse.tile as tile
from concourse import bacc
from concourse.bass_utils import run_bass_kernel_spmd
from concourse.masks import make_identity

F32 = mybir.dt.float32
F32R = mybir.dt.float32r

B = 2
S = 1024          # tokens per batch
H = 4096          # model dim
D = 128           # head dim
HQ = 4            # q heads per core
NT = B * S        # total tokens
SCALE = 1.0 / float(np.sqrt(D))
NEG = -1.0e30     # additive causal mask value (pre-scale)

QB = S // 128     # 8 q-blocks of 128 per batch
KC = S // 128     # 8 k-chunks of 128 per batch
HC = H // 128     # 32 contraction chunks for the projections


def r(ap):
    """View an fp32 AP as float32r for full-rate PE matmuls."""
    return ap.bitcast(F32R)


def build_program():
    nc = bacc.Bacc("TRN2", target_bir_lowering=False, debug=False, num_devices=8)

    xT = nc.dram_tensor("xT", [H, NT], F32, kind="ExternalInput").ap()
    wq = nc.dram_tensor("wq", [H, HQ * D], F32, kind="ExternalInput").ap()
    wkv = nc.dram_tensor("wkv", [H, 2 * D], F32, kind="ExternalInput").ap()
    wo = nc.dram_tensor("wo", [HQ * D, H], F32, kind="ExternalInput").ap()
    cosf = nc.dram_tensor("cosf", [128, S], F32, kind="ExternalInput").ap()
    sinf = nc.dram_tensor("sinf", [128, S], F32, kind="ExternalInput").ap()
    out = nc.dram_tensor("out", [NT, H], F32, kind="ExternalOutput").ap()

    wq_r = wq.rearrange("(n p) f -> p n f", p=128)     # [128, 32, 512]
    wkv_r = wkv.rearrange("(n p) f -> p n f", p=128)   # [128, 32, 256]
    wo_r = wo.rearrange("(n p) f -> p n f", p=128)     # [128, 4, 4096]

    with tile.TileContext(nc) as tc:
        with (
            tc.tile_pool(name="const", bufs=1) as const,
            tc.tile_pool(name="weights", bufs=1) as weights,
            tc.tile_pool(name="stream", bufs=4) as stream,
            tc.tile_pool(name="acts", bufs=1) as acts,
            tc.tile_pool(name="work", bufs=5) as work,
            tc.tile_pool(name="stats", bufs=16) as stats,
            tc.tile_pool(name="ps", bufs=8, space="PSUM") as pspool,
        ):
            # ---- constants -------------------------------------------------
            ident = const.tile([128, 128], F32)
            make_identity(nc, ident)

            maskadd = const.tile([128, 128], F32)
            nc.gpsimd.memset(maskadd, 0.0)
            # maskadd[p, f] = 0 where f <= p (valid causal), NEG above diagonal
            nc.gpsimd.affine_select(
                out=maskadd,
                in_=maskadd,
                compare_op=mybir.AluOpType.is_ge,
                fill=NEG,
                base=0,
                pattern=[[-1, 128]],
                channel_multiplier=1,
            )

            cosf_s = const.tile([128, S], F32)
            nc.sync.dma_start(out=cosf_s, in_=cosf)
            sinf_s = const.tile([128, S], F32)
            nc.sync.dma_start(out=sinf_s, in_=sinf)

            # ---- resident weights -----------------------------------------
            wq_s = weights.tile([128, HC, HQ * D], F32R)
            for i in range(4):
                nc.sync.dma_start(out=wq_s[:, i * 8:(i + 1) * 8, :],
                                  in_=wq_r[:, i * 8:(i + 1) * 8, :].bitcast(F32R))
            wkv_s = weights.tile([128, HC, 2 * D], F32R)
            for i in range(2):
                nc.sync.dma_start(out=wkv_s[:, i * 16:(i + 1) * 16, :],
                                  in_=wkv_r[:, i * 16:(i + 1) * 16, :].bitcast(F32R))

            for b in range(B):
                tok0 = b * S

                # ---- projections: qT/kT/vT = w.T @ x ----------------------
                qT = acts.tile([128, HQ, S], F32R, tag="qT")
                kT = acts.tile([128, S], F32R, tag="kT")
                vT = acts.tile([128, S], F32, tag="vT")

                for t in range(2):  # two 512-token chunks per batch
                    ts_ = slice(t * 512, (t + 1) * 512)
                    psq = [pspool.tile([128, 512], F32, tag="ps", name=f"psq{_d}")
                           for _d in range(HQ)]
                    psk = pspool.tile([128, 512], F32, tag="ps")
                    psv = pspool.tile([128, 512], F32, tag="ps")
                    for hc in range(HC):
                        xp = stream.tile([128, 512], F32R, tag="xp")
                        nc.sync.dma_start(
                            out=xp,
                            in_=xT[hc * 128:(hc + 1) * 128,
                                   tok0 + t * 512: tok0 + (t + 1) * 512].bitcast(F32R),
                        )
                        first, last = hc == 0, hc == HC - 1
                        for d in range(HQ):
                            nc.tensor.matmul(
                                psq[d],
                                r(wq_s[:, hc, d * 128:(d + 1) * 128]),
                                r(xp),
                                start=first, stop=last,
                            )
                        nc.tensor.matmul(psk, r(wkv_s[:, hc, 0:128]), r(xp),
                                         start=first, stop=last)
                        nc.tensor.matmul(psv, r(wkv_s[:, hc, 128:256]), r(xp),
                                         start=first, stop=last)
                    for d in range(HQ):
                        nc.scalar.copy(qT[:, d, ts_], psq[d])
                    nc.scalar.copy(kT[:, ts_], psk)
                    nc.scalar.copy(vT[:, ts_], psv)

                # ---- RoPE (halves live on different partitions; swap via
                # SBUF->SBUF DMA so every DVE op stays partition-aligned) ----
                def rope(dst):  # dst: [128, S] AP, in-place
                    scr = work.tile([128, S], F32R, tag="scr", bufs=1)
                    nc.sync.dma_start(out=scr[0:64, :], in_=dst[64:128, :])
                    nc.sync.dma_start(out=scr[64:128, :], in_=dst[0:64, :])
                    nc.vector.tensor_mul(dst[0:64, :], dst[0:64, :], cosf_s[0:64, :])
                    nc.vector.tensor_mul(scr[0:64, :], scr[0:64, :], sinf_s[0:64, :])
                    nc.vector.tensor_sub(dst[0:64, :], dst[0:64, :], scr[0:64, :])
                    nc.vector.tensor_mul(dst[64:128, :], dst[64:128, :], cosf_s[64:128, :])
                    nc.vector.tensor_mul(scr[64:128, :], scr[64:128, :], sinf_s[64:128, :])
                    nc.vector.tensor_add(dst[64:128, :], dst[64:128, :], scr[64:128, :])

                for hh in range(HQ):
                    rope(qT[:, hh, :])
                rope(kT)

                # ---- v natural [tok, d] via PE transpose of vT blocks ------
                vnat = acts.tile([128, KC, D], F32R, tag="vnat")
                for g in range(2):  # 4 blocks per psum slot
                    tp = pspool.tile([128, 512], F32, tag="ps")
                    for i in range(4):
                        kc = g * 4 + i
                        nc.tensor.transpose(
                            tp[:, i * 128:(i + 1) * 128],
                            vT[:, kc * 128:(kc + 1) * 128],
                            ident,
                        )
                    nc.vector.tensor_copy(vnat[:, g * 4:(g + 1) * 4, :], tp)

                # ---- attention per head ------------------------------------
                attnT = acts.tile([128, HQ, S], F32R, tag="attnT")
                for hh in range(HQ):
                    for qc in range(2):  # 512-wide q windows
                        probs_tiles = {}
                        for j in range(qc * 4, qc * 4 + 4):
                            kcols = (j + 1) * 128
                            nch = (kcols + 511) // 512
                            sc = []
                            for ch in range(nch):
                                cols = min(512, kcols - ch * 512)
                                ps = pspool.tile([128, 512], F32, tag="ps")
                                nc.tensor.matmul(
                                    ps[:, :cols],
                                    r(qT[:, hh, j * 128:(j + 1) * 128]),
                                    r(kT[:, ch * 512: ch * 512 + cols]),
                                    start=True, stop=True,
                                )
                                sc.append((ps, cols))
                            # additive causal mask on the diagonal block
                            dps, dcols = sc[-1]
                            off = j * 128 - (nch - 1) * 512
                            nc.vector.tensor_add(
                                dps[:, off:off + 128],
                                dps[:, off:off + 128],
                                maskadd,
                            )
                            # row max across chunks
                            mxs = []
                            for ps, cols in sc:
                                mx = stats.tile([128, 1], F32, tag="st")
                                nc.vector.tensor_reduce(
                                    mx, ps[:, :cols],
                                    axis=mybir.AxisListType.X,
                                    op=mybir.AluOpType.max,
                                )
                                mxs.append(mx)
                            mx = mxs[0]
                            if len(mxs) > 1:
                                mx2 = stats.tile([128, 1], F32, tag="st")
                                nc.vector.tensor_max(mx2, mxs[0], mxs[1])
                                mx = mx2
                            negm = stats.tile([128, 1], F32, tag="st")
                            nc.vector.tensor_scalar_mul(negm, mx, -SCALE)
                            # exp(scale*x - scale*max) with fused row-sum
                            probs = work.tile([128, S], F32, tag="probs", bufs=4)
                            dens = []
                            for ch, (ps, cols) in enumerate(sc):
                                den = stats.tile([128, 1], F32, tag="st")
                                nc.scalar.activation(
                                    probs[:, ch * 512: ch * 512 + cols],
                                    ps[:, :cols],
                                    mybir.ActivationFunctionType.Exp,
                                    bias=negm,
                                    scale=SCALE,
                                    accum_out=den,
                                )
                                dens.append(den)
                            den = dens[0]
                            if len(dens) > 1:
                                den2 = stats.tile([128, 1], F32, tag="st")
                                nc.vector.tensor_add(den2, dens[0], dens[1])
                                den = den2
                            rec = stats.tile([128, 1], F32, tag="st")
                            nc.vector.reciprocal(rec, den)
                            nc.vector.tensor_scalar_mul(
                                probs[:, :kcols], probs[:, :kcols], rec)
                            probs_tiles[j] = probs

                        # transpose probs into [k, q] layout for PV
                        probsT = work.tile([128, KC, 512], F32R, tag="probsT",
                                           bufs=1)
                        for kc in range(qc * 4 + 4):
                            jlo = max(qc * 4, kc)
                            tp = pspool.tile([128, 512], F32, tag="ps")
                            for j in range(jlo, qc * 4 + 4):
                                rel = j - qc * 4
                                nc.tensor.transpose(
                                    tp[:, rel * 128:(rel + 1) * 128],
                                    probs_tiles[j][:, kc * 128:(kc + 1) * 128],
                                    ident,
                                )
                            lo = (jlo - qc * 4) * 128
                            nc.vector.tensor_copy(
                                probsT[:, kc, lo:512], tp[:, lo:512])

                        # PV: attnT[d, q] += v[k, d].T-free accumulation
                        pa = pspool.tile([128, 512], F32, tag="ps")
                        kcs = list(range(qc * 4 + 4))
                        for i, kc in enumerate(kcs):
                            a = max(0, kc * 128 - qc * 512)
                            nc.tensor.matmul(
                                pa[:, a:512],
                                r(vnat[:, kc, :]),
                                r(probsT[:, kc, a:512]),
                                start=(i == 0), stop=(i == len(kcs) - 1),
                            )
                        nc.scalar.copy(attnT[:, hh, qc * 512:(qc + 1) * 512], pa)

                # ---- output projection: out[tok, :] += attnT.T @ wo --------
                for ncol in range(8):  # 512-wide output column chunks
                    wps = []
                    for d in range(HQ):
                        wp = stream.tile([128, 512], F32R, tag="wo")
                        nc.sync.dma_start(
                            out=wp,
                            in_=wo_r[:, d, ncol * 512:(ncol + 1) * 512].bitcast(F32R))
                        wps.append(wp)
                    for tb in range(QB):
                        po = pspool.tile([128, 512], F32, tag="ps")
                        for d in range(HQ):
                            nc.tensor.matmul(
                                po,
                                r(attnT[:, d, tb * 128:(tb + 1) * 128]),
                                r(wps[d]),
                                start=(d == 0), stop=(d == HQ - 1),
                            )
                        ev = work.tile([128, 512], F32, tag="ev", bufs=2)
                        nc.scalar.copy(ev, po)
                        nc.sync.dma_start(
                            out=out[tok0 + tb * 128: tok0 + (tb + 1) * 128,
                                    ncol * 512:(ncol + 1) * 512],
                            in_=ev,
                        )

    nc.compile()
    return nc


_NC = None


def _get_nc():
    global _NC
    if _NC is None:
        _NC = build_program()
    return _NC


def make_in_maps(x, wq, wk, wv, wo, freqs_cos, freqs_sin):
    x = np.asarray(x, np.float32)
    xT = np.ascontiguousarray(x.reshape(NT, H).T)
    cosT = np.asarray(freqs_cos, np.float32).T
    sinT = np.asarray(freqs_sin, np.float32).T
    cosf = np.ascontiguousarray(np.concatenate([cosT, cosT], 0))
    sinf = np.ascontiguousarray(np.concatenate([sinT, sinT], 0))
    wq = np.asarray(wq, np.float32)
    wk = np.asarray(wk, np.float32)
    wv = np.asarray(wv, np.float32)
    wo = np.asarray(wo, np.float32)
    in_maps = []
    for c in range(8):
        in_maps.append({
            "xT": xT,
            "wq": np.ascontiguousarray(wq[:, c * 512:(c + 1) * 512]),
            "wkv": np.ascontiguousarray(
                np.concatenate([wk[:, c * 128:(c + 1) * 128],
                                wv[:, c * 128:(c + 1) * 128]], axis=1)),
            "wo": np.ascontiguousarray(wo[c * 512:(c + 1) * 512, :]),
            "cosf": cosf,
            "sinf": sinf,
        })
    return in_maps


def kernel(x, wq, wk, wv, wo, freqs_cos, freqs_sin, start_pos=0, **_):
    nc = _get_nc()
    in_maps = make_in_maps(x, wq, wk, wv, wo, freqs_cos, freqs_sin)
    res = run_bass_kernel_spmd(nc, in_maps, list(range(8)))
    acc = res.results[0]["out"].astype(np.float32)
    for c in range(1, 8):
        acc = acc + res.results[c]["out"]
    return acc.reshape(B, S, H)



# revision 6
# speedup vs baseline: 2.1312x; 2.1312x over previous
"""Trainium2 Bass kernel for fused Llama attention (nn_LlamaAttentionFused).

Reference computation (B=2, S=1024, H=4096, 32 Q heads, 8 KV heads, D=128):
    xq = x @ wq; xk = x @ wk; xv = x @ wv
    rope(xq, xk); causal GQA flash attention; out = attn @ wo

Sharding: 8-way tensor parallel over heads. Core c owns Q heads 4c..4c+3 and
KV head c (GQA groups stay together). Each core computes a full-shape partial
output (its heads' contribution through wo); the host sums the 8 partials.

v2 design notes:
  - Projections in bf16 (x, wq, wkv) with fp32 PSUM accumulation; qT/kT kept
    fp32 so attention scores stay accurate. RoPE in fp32 on DVE with the sign
    of the sin term folded into the constant (3 tensor ops per head instead
    of 6), overlapped with the other batch's projection matmuls.
  - Attention runs in transposed layout: scoresT[k, q] = kT_blk.T @ qT, exp
    with no max subtraction (scores are bounded, ~|s| < 20), causal mask as a
    post-exp 0/1 multiply on the diagonal blocks only. PV uses the probs
    block as the stationary operand and v-natural (with a ones column
    appended) as the moving operand, so the softmax denominator lands in the
    PV psum as column 128 -- per-partition, where tensor_scalar can use it.
  - One 128x128 PE transpose per q-block turns normalized attn back into
    attnT for the output projection (64 transposes/core vs 304 in v1).
  - wo resident in SBUF (bf16); output stored bf16 and summed on host.

Device-side layouts (per core):
    xT   [4096, 2048] bf16  x transposed on host (tokens = 2 batches x 1024)
    wq   [4096, 512]  bf16
    wkv  [4096, 256]  bf16  wk|wv column-concat
    wo   [512, 4096]  bf16
    cosf [128, 1024]  f32   freqs_cos.T stacked twice on partitions
    sinf [128, 1024]  f32   [-freqs_sin.T ; +freqs_sin.T]
    out  [2048, 4096] bf16  partial output
"""

import numpy as np
import ml_dtypes

import concourse.bass as bass
import concourse.mybir as mybir
import concourse.tile as tile
from concourse import bacc
from concourse.bass_utils import run_bass_kernel_spmd
from concourse.masks import make_identity

F32 = mybir.dt.float32
F32R = mybir.dt.float32r
BF16 = mybir.dt.bfloat16

B = 2
S = 1024          # tokens per batch
H = 4096          # model dim
D = 128           # head dim
HQ = 4            # q heads per core
NT = B * S        # total tokens
SCALE = 1.0 / float(np.sqrt(D))

QB = S // 128     # 8 q-blocks of 128 per batch
KC = S // 128     # 8 k-chunks of 128 per batch
HC = H // 128     # 32 contraction chunks for the projections
VW = 132          # vnat row width: 128 v dims + ones col + 3 zero pad


def r(ap):
    """View an fp32 AP as float32r for full-rate PE matmuls."""
    return ap.bitcast(F32R)


def build_program():
    nc = bacc.Bacc("TRN2", target_bir_lowering=False, debug=False, num_devices=8)

    xT = nc.dram_tensor("xT", [H, NT], BF16, kind="ExternalInput").ap()
    wq = nc.dram_tensor("wq", [H, HQ * D], BF16, kind="ExternalInput").ap()
    wkv = nc.dram_tensor("wkv", [H, 2 * D], BF16, kind="ExternalInput").ap()
    wo = nc.dram_tensor("wo", [HQ * D, H], BF16, kind="ExternalInput").ap()
    cosf = nc.dram_tensor("cosf", [128, S], F32, kind="ExternalInput").ap()
    sinf = nc.dram_tensor("sinf", [128, S], F32, kind="ExternalInput").ap()
    out = nc.dram_tensor("out", [NT, H], BF16, kind="ExternalOutput").ap()

    wq_r = wq.rearrange("(n p) f -> p n f", p=128)     # [128, 32, 512]
    wkv_r = wkv.rearrange("(n p) f -> p n f", p=128)   # [128, 32, 256]
    wo_r = wo.rearrange("(n p) f -> p n f", p=128)     # [128, 4, 4096]

    with tile.TileContext(nc) as tc:
        with (
            tc.tile_pool(name="const", bufs=1) as const,
            tc.tile_pool(name="weights", bufs=1) as weights,
            tc.tile_pool(name="stream", bufs=4) as stream,
            tc.tile_pool(name="acts", bufs=1) as acts,
            tc.tile_pool(name="work", bufs=2) as work,
            tc.tile_pool(name="stats", bufs=16) as stats,
        ):
            # ---- constants -------------------------------------------------
            ident = const.tile([128, 128], BF16)
            make_identity(nc, ident)

            # maskT01[k, q] = 1 where k <= q (valid causal in [k,q] layout)
            maskT01 = const.tile([128, 128], BF16)
            nc.gpsimd.memset(maskT01, 1.0)
            nc.gpsimd.affine_select(
                out=maskT01,
                in_=maskT01,
                compare_op=mybir.AluOpType.is_ge,
                fill=0.0,
                base=0,
                pattern=[[1, 128]],       # expr = -p + f >= 0 -> keep
                channel_multiplier=-1,
            )

            cosf_s = const.tile([128, S], F32)
            nc.sync.dma_start(out=cosf_s, in_=cosf)
            sinf_s = const.tile([128, S], F32)
            nc.sync.dma_start(out=sinf_s, in_=sinf)

            # ---- resident weights -----------------------------------------
            wq_s = weights.tile([128, HC, HQ * D], BF16)
            for i in range(4):
                nc.sync.dma_start(out=wq_s[:, i * 8:(i + 1) * 8, :],
                                  in_=wq_r[:, i * 8:(i + 1) * 8, :])
            wkv_s = weights.tile([128, HC, 2 * D], BF16)
            for i in range(2):
                nc.sync.dma_start(out=wkv_s[:, i * 16:(i + 1) * 16, :],
                                  in_=wkv_r[:, i * 16:(i + 1) * 16, :])
            wo_s = weights.tile([128, HQ, H], BF16)
            for i in range(4):
                nc.sync.dma_start(out=wo_s[:, i, :], in_=wo_r[:, i, :])

            # ---- per-batch activations ------------------------------------
            qT = [acts.tile([128, HQ, S], F32R, tag=f"qT{b}", name=f"qT{b}")
                  for b in range(B)]
            kT = [acts.tile([128, S], F32R, tag=f"kT{b}", name=f"kT{b}")
                  for b in range(B)]
            vnat = [acts.tile([128, KC, VW], BF16, tag=f"vn{b}", name=f"vn{b}")
                    for b in range(B)]
            attnT = [acts.tile([128, HQ, S], BF16, tag=f"aT{b}", name=f"aT{b}")
                     for b in range(B)]

            # ================= Phase 1: projections + rope =================
            with tc.tile_pool(name="pproj", bufs=1, space="PSUM") as psp:
                for b in range(B):
                    tok0 = b * S
                    for t in range(2):  # two 512-token chunks per batch
                        psq = [psp.tile([128, 512], F32, tag="pj", bufs=6,
                                        name=f"psq{b}{t}{d}") for d in range(HQ)]
                        psk = psp.tile([128, 512], F32, tag="pj", bufs=6)
                        psv = psp.tile([128, 512], F32, tag="pj", bufs=6)
                        for hc in range(HC):
                            xp = stream.tile([128, 512], BF16, tag="xp")
                            nc.sync.dma_start(
                                out=xp,
                                in_=xT[hc * 128:(hc + 1) * 128,
                                       tok0 + t * 512: tok0 + (t + 1) * 512],
                            )
                            first, last = hc == 0, hc == HC - 1
                            for d in range(HQ):
                                nc.tensor.matmul(
                                    psq[d],
                                    wq_s[:, hc, d * 128:(d + 1) * 128],
                                    xp,
                                    start=first, stop=last,
                                )
                            nc.tensor.matmul(psk, wkv_s[:, hc, 0:128], xp,
                                             start=first, stop=last)
                            nc.tensor.matmul(psv, wkv_s[:, hc, 128:256], xp,
                                             start=first, stop=last)
                        ts_ = slice(t * 512, (t + 1) * 512)
                        for d in range(HQ):
                            nc.scalar.copy(qT[b][:, d, ts_], psq[d])
                        nc.scalar.copy(kT[b][:, ts_], psk)
                        # v: transpose to natural [tok, d] via PE
                        vT_sb = work.tile([128, 512], BF16, tag="vT", bufs=2)
                        nc.scalar.copy(vT_sb, psv)
                        ptr = psp.tile([128, 512], BF16, tag="trv", bufs=1,
                                       padded_shape=[128, 1024])
                        for i in range(4):
                            nc.tensor.transpose(
                                ptr[:, i * 128:(i + 1) * 128],
                                vT_sb[:, i * 128:(i + 1) * 128],
                                ident,
                            )
                        nc.vector.tensor_copy(
                            vnat[b][:, t * 4:(t + 1) * 4, 0:128],
                            ptr.rearrange("p (n f) -> p n f", n=4),
                        )
                    # ones column for the softmax denominator; zero pad
                    nc.gpsimd.memset(vnat[b][:, :, 128:129], 1.0)
                    nc.gpsimd.memset(vnat[b][:, :, 129:VW], 0.0)

                    # rope on DVE (overlaps the next batch's projections)
                    def rope(dst):  # [128, S] f32, in place
                        scr = work.tile([128, S], F32R, tag="scr", bufs=2)
                        nc.sync.dma_start(out=scr[0:64, :], in_=dst[64:128, :])
                        nc.sync.dma_start(out=scr[64:128, :], in_=dst[0:64, :])
                        nc.vector.tensor_mul(dst, dst, cosf_s)
                        nc.vector.tensor_mul(scr, scr, sinf_s)
                        nc.vector.tensor_add(dst, dst, scr)

                    for hh in range(HQ):
                        rope(qT[b][:, hh, :])
                    rope(kT[b])

            # ============ Phase 2+3: attention, output projection ==========
            with tc.tile_pool(name="pattn", bufs=1, space="PSUM") as psa:
                for b in range(B):
                    for hh in range(HQ):
                        PT = work.tile([128, KC, S], BF16, tag="pt", bufs=2)
                        for kc in range(KC):
                            qlo = kc * 128
                            # QK^T transposed: scoresT[k, q] (k on partitions)
                            spans = ([(qlo, 512), (512, S)] if qlo < 512
                                     else [(qlo, S)])
                            for (a, e) in spans:
                                ps = psa.tile([128, 512], F32, tag="qk", bufs=3)
                                nc.tensor.matmul(
                                    ps[:, :e - a],
                                    r(kT[b][:, qlo:qlo + 128]),
                                    r(qT[b][:, hh, a:e]),
                                    start=True, stop=True,
                                )
                                # exp(scale*s), no max subtraction (bounded)
                                nc.scalar.activation(
                                    PT[:, kc, a:e],
                                    ps[:, :e - a],
                                    mybir.ActivationFunctionType.Exp,
                                    scale=SCALE,
                                )
                            # causal 0/1 mask on the diagonal block
                            nc.vector.tensor_mul(
                                PT[:, kc, qlo:qlo + 128],
                                PT[:, kc, qlo:qlo + 128],
                                maskT01,
                            )
                            # PV for q-block qb=kc: attn_nat[q, d] (+den col)
                            qb = kc
                            pv = psa.tile([128, VW], F32, tag="pv", bufs=2,
                                          padded_shape=[128, 512])
                            for kc2 in range(qb + 1):
                                nc.tensor.matmul(
                                    pv,
                                    PT[:, kc2, qb * 128:(qb + 1) * 128],
                                    vnat[b][:, kc2, :],
                                    start=(kc2 == 0), stop=(kc2 == qb),
                                )
                            rec = stats.tile([128, 1], F32, tag="st")
                            nc.vector.reciprocal(rec, pv[:, 128:129])
                            an = work.tile([128, 128], BF16, tag="an", bufs=2)
                            nc.vector.tensor_scalar_mul(an, pv[:, 0:128], rec)
                            # transpose back to attnT[d, q] for the out-proj
                            ptr = psa.tile([128, 128], BF16, tag="tr2", bufs=1,
                                           padded_shape=[128, 1024])
                            nc.tensor.transpose(ptr, an, ident)
                            nc.vector.tensor_copy(
                                attnT[b][:, hh, qb * 128:(qb + 1) * 128], ptr)

                    # ---- output projection for batch b --------------------
                    tok0 = b * S
                    for tb in range(QB):
                        for ncol in range(8):
                            po = psa.tile([128, 512], F32, tag="op", bufs=2)
                            for d in range(HQ):
                                nc.tensor.matmul(
                                    po,
                                    attnT[b][:, d, tb * 128:(tb + 1) * 128],
                                    wo_s[:, d, ncol * 512:(ncol + 1) * 512],
                                    start=(d == 0), stop=(d == HQ - 1),
                                )
                            ev = work.tile([128, 512], BF16, tag="ev", bufs=4)
                            if ncol % 2 == 0:
                                nc.scalar.copy(ev, po)
                            else:
                                nc.vector.tensor_copy(ev, po)
                            nc.sync.dma_start(
                                out=out[tok0 + tb * 128: tok0 + (tb + 1) * 128,
                                        ncol * 512:(ncol + 1) * 512],
                                in_=ev,
                            )

    nc.compile()
    return nc


_NC = None


def _get_nc():
    global _NC
    if _NC is None:
        _NC = build_program()
    return _NC


def make_in_maps(x, wq, wk, wv, wo, freqs_cos, freqs_sin):
    bf = ml_dtypes.bfloat16
    x = np.asarray(x, np.float32)
    xT = np.ascontiguousarray(x.reshape(NT, H).T.astype(bf))
    cosT = np.asarray(freqs_cos, np.float32).T
    sinT = np.asarray(freqs_sin, np.float32).T
    cosf = np.ascontiguousarray(np.concatenate([cosT, cosT], 0))
    sinf = np.ascontiguousarray(np.concatenate([-sinT, sinT], 0))
    wq = np.asarray(wq, np.float32).astype(bf)
    wk = np.asarray(wk, np.float32).astype(bf)
    wv = np.asarray(wv, np.float32).astype(bf)
    wo = np.asarray(wo, np.float32).astype(bf)
    in_maps = []
    for c in range(8):
        in_maps.append({
            "xT": xT,
            "wq": np.ascontiguousarray(wq[:, c * 512:(c + 1) * 512]),
            "wkv": np.ascontiguousarray(
                np.concatenate([wk[:, c * 128:(c + 1) * 128],
                                wv[:, c * 128:(c + 1) * 128]], axis=1)),
            "wo": np.ascontiguousarray(wo[c * 512:(c + 1) * 512, :]),
            "cosf": cosf,
            "sinf": sinf,
        })
    return in_maps


def kernel(x, wq, wk, wv, wo, freqs_cos, freqs_sin, start_pos=0, **_):
    nc = _get_nc()
    in_maps = make_in_maps(x, wq, wk, wv, wo, freqs_cos, freqs_sin)
    res = run_bass_kernel_spmd(nc, in_maps, list(range(8)))
    acc = res.results[0]["out"].astype(np.float32)
    for c in range(1, 8):
        acc = acc + res.results[c]["out"].astype(np.float32)
    return acc.reshape(B, S, H)


# revision 9
# speedup vs baseline: 2.2941x; 1.0764x over previous
"""Trainium2 Bass kernel for fused Llama attention (nn_LlamaAttentionFused).

Reference computation (B=2, S=1024, H=4096, 32 Q heads, 8 KV heads, D=128):
    xq = x @ wq; xk = x @ wk; xv = x @ wv
    rope(xq, xk); causal GQA flash attention; out = attn @ wo

Sharding: 8-way tensor parallel over heads. Core c owns Q heads 4c..4c+3 and
KV head c (GQA groups stay together). Each core computes a full-shape partial
output (its heads' contribution through wo); the host sums the 8 partials.

v3 design notes:
  - Projections in bf16 (x, wq, wkv) with fp32 PSUM accumulation. q/k stored
    bf16; RoPE in bf16 on DVE (2x mode) with the sign of the sin term folded
    into the constant (3 tensor ops per head), overlapped with the next
    batch's projection matmuls. wo loads are emitted after batch 0's
    projections so they don't delay the first matmuls.
  - Attention in transposed layout: scoresT[k, q] = kT_blk.T @ qT as a single
    N<=1024 bf16 matmul per k-chunk into a bf16 PSUM bank; exp with no max
    subtraction (scores bounded); causal mask as post-exp 0/1 multiply on the
    diagonal block. PV uses the probs block as stationary and v-natural with
    a ones column as moving, so the softmax denominator lands per-partition
    in the PV psum; normalize via tensor_scalar; one PE transpose per qb.
  - Attention and output projection are fused per batch at q-block
    granularity: after PV of q-block qb for all 4 heads, the out-proj for
    that token block runs immediately, keeping the PE dense while ScalarE
    computes the next exps.

Device-side layouts (per core):
    xT   [4096, 2048] bf16  x transposed on host (tokens = 2 batches x 1024)
    wq   [4096, 512]  bf16
    wkv  [4096, 256]  bf16  wk|wv column-concat
    wo   [512, 4096]  bf16
    cosf [128, 1024]  bf16  freqs_cos.T stacked twice on partitions
    sinf [128, 1024]  bf16  [-freqs_sin.T ; +freqs_sin.T]
    out  [2048, 4096] bf16  partial output
"""

import numpy as np
import ml_dtypes

import concourse.bass as bass
import concourse.mybir as mybir
import concourse.tile as tile
from concourse import bacc
from concourse.bass_utils import run_bass_kernel_spmd
from concourse.masks import make_identity

F32 = mybir.dt.float32
F32R = mybir.dt.float32r
BF16 = mybir.dt.bfloat16

B = 2
S = 1024          # tokens per batch
H = 4096          # model dim
D = 128           # head dim
HQ = 4            # q heads per core
NT = B * S        # total tokens
SCALE = 1.0 / float(np.sqrt(D))

QB = S // 128     # 8 q-blocks of 128 per batch
KC = S // 128     # 8 k-chunks of 128 per batch
HC = H // 128     # 32 contraction chunks for the projections
VW = 132          # vnat row width: 128 v dims + ones col + 3 zero pad


def build_program():
    nc = bacc.Bacc("TRN2", target_bir_lowering=False, debug=False, num_devices=8)

    xT = nc.dram_tensor("xT", [H, NT], BF16, kind="ExternalInput").ap()
    wq = nc.dram_tensor("wq", [H, HQ * D], BF16, kind="ExternalInput").ap()
    wkv = nc.dram_tensor("wkv", [H, 2 * D], BF16, kind="ExternalInput").ap()
    wo = nc.dram_tensor("wo", [HQ * D, H], BF16, kind="ExternalInput").ap()
    cosf = nc.dram_tensor("cosf", [128, S], BF16, kind="ExternalInput").ap()
    sinf = nc.dram_tensor("sinf", [128, S], BF16, kind="ExternalInput").ap()
    out = nc.dram_tensor("out", [NT, H], BF16, kind="ExternalOutput").ap()

    wq_r = wq.rearrange("(n p) f -> p n f", p=128)     # [128, 32, 512]
    wkv_r = wkv.rearrange("(n p) f -> p n f", p=128)   # [128, 32, 256]
    wo_r = wo.rearrange("(n p) f -> p n f", p=128)     # [128, 4, 4096]

    with tile.TileContext(nc) as tc:
        with (
            tc.tile_pool(name="const", bufs=1) as const,
            tc.tile_pool(name="weights", bufs=1) as weights,
            tc.tile_pool(name="stream", bufs=4) as stream,
            tc.tile_pool(name="acts", bufs=1) as acts,
            tc.tile_pool(name="work", bufs=2) as work,
            tc.tile_pool(name="stats", bufs=16) as stats,
        ):
            # ---- constants -------------------------------------------------
            ident = const.tile([128, 128], BF16)
            make_identity(nc, ident)

            # maskT01[k, q] = 1 where k <= q (valid causal in [k,q] layout)
            maskT01 = const.tile([128, 128], BF16)
            nc.gpsimd.memset(maskT01, 1.0)
            nc.gpsimd.affine_select(
                out=maskT01,
                in_=maskT01,
                compare_op=mybir.AluOpType.is_ge,
                fill=0.0,
                base=0,
                pattern=[[1, 128]],       # expr = -p + f >= 0 -> keep
                channel_multiplier=-1,
            )

            cosf_s = const.tile([128, S], BF16)
            nc.sync.dma_start(out=cosf_s, in_=cosf)
            sinf_s = const.tile([128, S], BF16)
            nc.sync.dma_start(out=sinf_s, in_=sinf)

            # ---- resident weights (wo loaded later, after proj b0) --------
            wq_s = weights.tile([128, HC, HQ * D], BF16)
            for i in range(8):
                nc.sync.dma_start(out=wq_s[:, i * 4:(i + 1) * 4, :],
                                  in_=wq_r[:, i * 4:(i + 1) * 4, :])
            wkv_s = weights.tile([128, HC, 2 * D], BF16)
            for i in range(4):
                nc.sync.dma_start(out=wkv_s[:, i * 8:(i + 1) * 8, :],
                                  in_=wkv_r[:, i * 8:(i + 1) * 8, :])
            wo_s = weights.tile([128, HQ, H], BF16)

            # ---- per-batch activations ------------------------------------
            qT = [acts.tile([128, HQ, S], BF16, tag=f"qT{b}", name=f"qT{b}")
                  for b in range(B)]
            kT = [acts.tile([128, S], BF16, tag=f"kT{b}", name=f"kT{b}")
                  for b in range(B)]
            vnat = [acts.tile([128, KC, VW], BF16, tag=f"vn{b}", name=f"vn{b}")
                    for b in range(B)]

            # ================= Phase 1: projections + rope =================
            with tc.tile_pool(name="pproj", bufs=1, space="PSUM") as psp:
                for b in range(B):
                    tok0 = b * S
                    for t in range(2):  # two 512-token chunks per batch
                        psq = [psp.tile([128, 512], F32, tag="pj", bufs=6,
                                        name=f"psq{b}{t}{d}") for d in range(HQ)]
                        psk = psp.tile([128, 512], F32, tag="pj", bufs=6)
                        psv = psp.tile([128, 512], F32, tag="pj", bufs=6)
                        for hc in range(HC):
                            xp = stream.tile([128, 512], BF16, tag="xp")
                            nc.sync.dma_start(
                                out=xp,
                                in_=xT[hc * 128:(hc + 1) * 128,
                                       tok0 + t * 512: tok0 + (t + 1) * 512],
                            )
                            first, last = hc == 0, hc == HC - 1
                            for d in range(HQ):
                                nc.tensor.matmul(
                                    psq[d],
                                    wq_s[:, hc, d * 128:(d + 1) * 128],
                                    xp,
                                    start=first, stop=last,
                                )
                            nc.tensor.matmul(psk, wkv_s[:, hc, 0:128], xp,
                                             start=first, stop=last)
                            nc.tensor.matmul(psv, wkv_s[:, hc, 128:256], xp,
                                             start=first, stop=last)
                        ts_ = slice(t * 512, (t + 1) * 512)
                        for d in range(HQ):
                            nc.scalar.copy(qT[b][:, d, ts_], psq[d])
                        nc.scalar.copy(kT[b][:, ts_], psk)
                        # v: transpose to natural [tok, d] via PE
                        vT_sb = work.tile([128, 512], BF16, tag="vT", bufs=2)
                        nc.scalar.copy(vT_sb, psv)
                        ptr = psp.tile([128, 512], BF16, tag="trv", bufs=1,
                                       padded_shape=[128, 1024])
                        for i in range(4):
                            nc.tensor.transpose(
                                ptr[:, i * 128:(i + 1) * 128],
                                vT_sb[:, i * 128:(i + 1) * 128],
                                ident,
                            )
                        nc.vector.tensor_copy(
                            vnat[b][:, t * 4:(t + 1) * 4, 0:128],
                            ptr.rearrange("p (n f) -> p n f", n=4),
                        )
                    # ones column for the softmax denominator; zero pad
                    nc.gpsimd.memset(vnat[b][:, :, 128:129], 1.0)
                    nc.gpsimd.memset(vnat[b][:, :, 129:VW], 0.0)

                    if b == 0:
                        # wo loads ride behind the proj-b0 xp stream; needed
                        # only at the first fused out-proj, ~170us later.
                        for i in range(8):
                            nc.sync.dma_start(out=wo_s[:, i // 2,
                                                       (i % 2) * 2048:
                                                       (i % 2 + 1) * 2048],
                                              in_=wo_r[:, i // 2,
                                                       (i % 2) * 2048:
                                                       (i % 2 + 1) * 2048])

                    # rope on DVE (overlaps the next batch's projections)
                    def rope(dst):  # [128, S] bf16, in place
                        scr = work.tile([128, S], BF16, tag="scr", bufs=2)
                        nc.sync.dma_start(out=scr[0:64, :], in_=dst[64:128, :])
                        nc.sync.dma_start(out=scr[64:128, :], in_=dst[0:64, :])
                        nc.vector.tensor_mul(dst, dst, cosf_s)
                        nc.vector.tensor_mul(scr, scr, sinf_s)
                        nc.vector.tensor_add(dst, dst, scr)

                    for hh in range(HQ):
                        rope(qT[b][:, hh, :])
                    rope(kT[b])

            # ===== Phase 2: fused attention + output projection ============
            with tc.tile_pool(name="pattn", bufs=1, space="PSUM") as psa:
                for b in range(B):
                    tok0 = b * S
                    PT = [work.tile([128, KC, S], BF16, tag=f"pt{hh}", bufs=1,
                                    name=f"PT{hh}") for hh in range(HQ)]
                    for kc in range(KC):
                        qlo = kc * 128
                        # QK^T transposed for all 4 heads at this k-chunk
                        spans = ([(qlo, 512), (512, S)] if qlo < 512
                                 else [(qlo, S)])
                        for hh in range(HQ):
                            for (a, e) in spans:
                                ps = psa.tile([128, 512], F32, tag="qk",
                                              bufs=3, name="psqk")
                                nc.tensor.matmul(
                                    ps[:, :e - a],
                                    kT[b][:, qlo:qlo + 128],
                                    qT[b][:, hh, a:e],
                                    start=True, stop=True,
                                )
                                nc.scalar.activation(
                                    PT[hh][:, kc, a:e],
                                    ps[:, :e - a],
                                    mybir.ActivationFunctionType.Exp,
                                    scale=SCALE,
                                )
                            nc.vector.tensor_mul(
                                PT[hh][:, kc, qlo:qlo + 128],
                                PT[hh][:, kc, qlo:qlo + 128],
                                maskT01,
                            )
                        # PV for q-block qb=kc, all heads -> attnT block
                        qb = kc
                        aT = work.tile([128, HQ, 128], BF16, tag="aT", bufs=2)
                        for hh in range(HQ):
                            pv = psa.tile([128, VW], F32, tag="pv", bufs=2,
                                          padded_shape=[128, 512], name="pspv")
                            for kc2 in range(qb + 1):
                                nc.tensor.matmul(
                                    pv,
                                    PT[hh][:, kc2, qb * 128:(qb + 1) * 128],
                                    vnat[b][:, kc2, :],
                                    start=(kc2 == 0), stop=(kc2 == qb),
                                )
                            rec = stats.tile([128, 1], F32, tag="st")
                            nc.vector.reciprocal(rec, pv[:, 128:129])
                            an = work.tile([128, 128], BF16, tag="an", bufs=2)
                            nc.vector.tensor_scalar_mul(an, pv[:, 0:128], rec)
                            ptr = psa.tile([128, 128], BF16, tag="tr2", bufs=1,
                                           padded_shape=[128, 1024], name="pstr")
                            nc.tensor.transpose(ptr, an, ident)
                            nc.vector.tensor_copy(aT[:, hh, :], ptr)
                        # out-proj for token block tb=qb
                        tb = qb
                        for ncol in range(8):
                            po = psa.tile([128, 512], F32, tag="op", bufs=2,
                                          name="psop")
                            for d in range(HQ):
                                nc.tensor.matmul(
                                    po,
                                    aT[:, d, :],
                                    wo_s[:, d, ncol * 512:(ncol + 1) * 512],
                                    start=(d == 0), stop=(d == HQ - 1),
                                )
                            ev = work.tile([128, 512], BF16, tag="ev", bufs=4)
                            if ncol % 2 == 0:
                                nc.scalar.copy(ev, po)
                            else:
                                nc.vector.tensor_copy(ev, po)
                            nc.sync.dma_start(
                                out=out[tok0 + tb * 128: tok0 + (tb + 1) * 128,
                                        ncol * 512:(ncol + 1) * 512],
                                in_=ev,
                            )

    nc.compile()
    return nc


_NC = None


def _get_nc():
    global _NC
    if _NC is None:
        _NC = build_program()
    return _NC


def make_in_maps(x, wq, wk, wv, wo, freqs_cos, freqs_sin):
    bf = ml_dtypes.bfloat16
    x = np.asarray(x, np.float32)
    xT = np.ascontiguousarray(x.reshape(NT, H).T.astype(bf))
    cosT = np.asarray(freqs_cos, np.float32).T
    sinT = np.asarray(freqs_sin, np.float32).T
    cosf = np.ascontiguousarray(np.concatenate([cosT, cosT], 0).astype(bf))
    sinf = np.ascontiguousarray(np.concatenate([-sinT, sinT], 0).astype(bf))
    wq = np.asarray(wq, np.float32).astype(bf)
    wk = np.asarray(wk, np.float32).astype(bf)
    wv = np.asarray(wv, np.float32).astype(bf)
    wo = np.asarray(wo, np.float32).astype(bf)
    in_maps = []
    for c in range(8):
        in_maps.append({
            "xT": xT,
            "wq": np.ascontiguousarray(wq[:, c * 512:(c + 1) * 512]),
            "wkv": np.ascontiguousarray(
                np.concatenate([wk[:, c * 128:(c + 1) * 128],
                                wv[:, c * 128:(c + 1) * 128]], axis=1)),
            "wo": np.ascontiguousarray(wo[c * 512:(c + 1) * 512, :]),
            "cosf": cosf,
            "sinf": sinf,
        })
    return in_maps


def kernel(x, wq, wk, wv, wo, freqs_cos, freqs_sin, start_pos=0, **_):
    nc = _get_nc()
    in_maps = make_in_maps(x, wq, wk, wv, wo, freqs_cos, freqs_sin)
    res = run_bass_kernel_spmd(nc, in_maps, list(range(8)))
    acc = res.results[0]["out"].astype(np.float32)
    for c in range(1, 8):
        acc = acc + res.results[c]["out"].astype(np.float32)
    return acc.reshape(B, S, H)
